# revision 25
# baseline (speedup 1.0000x reference)
"""Trainium2 Bass kernel for nn_BehaviorVelocity (velocity-driven swap sim + smoothing).

Sharding: data-parallel over batch B=16 across 8 cores (2 images/core, no collectives).

Layout per 512x512 image: partition p holds rows 4p..4p+3 as free-dim "slots".
Padded field = [128, 6 slots, 514 cols]:
  slot 0 = row 4p-1 (y-halo lo), slots 1..4 = rows 4p..4p+3, slot 5 = row 4p+4 (y-halo hi)
  col 0 = x=511 (wrap), cols 1..512 = x=0..511, col 513 = x=0 (wrap)
y-halos: partition-shift SBUF DMAs (+1-row torus wrap DMA). x-halos: tiny strided copies.
All spatial shifts then become free-dim AP offsets (compute ops must start at partition 0).

Channels 3,4 (vy,vx) stay f32 (decision precision). Payload channels are fp8(e4m3)
packed into containers so each copy_predicated moves more channels per cycle:
  wordA int32 = (ch1, ch2, ch5, ch6) fp8 bytes
  wordB uint16 = (ch0, ch7) fp8 bytes; ch0 (element id) is byte 0, compared as u8
  (0.0 -> 0x00, 1.0 -> 0x38).

Sector selection replicates floor(8*arccos-angle+0.5) via threshold compares in the
squared domain:  vx <= K*(mag+0.001)  <=>  (vx<=0) or (vx^2 <= K^2*magp2)  with
magp2 = m2 + 0.002*mag + 1e-6, so the ACT-sqrt LUT error only enters the tiny
0.002*mag term (~5e-8 boundary shift instead of ~3e-5).
"""

import sys

sys.path.insert(0, "/opt/trn_rl_repo")

import numpy as np

import concourse.bacc as bacc
import concourse.mybir as mybir
from concourse.tile import TileContext
from concourse.bass_utils import run_bass_kernel_spmd

dt = mybir.dt
Alu = mybir.AluOpType
Act = mybir.ActivationFunctionType

P = 128          # partitions
S = 4            # row-slots per partition (512 rows / 128)
W = 512
Wp = W + 2       # 514 with x-halo cols
NB = 2           # batch images per core
NCORES = 8

_DY = [0, 1, 1, 1, 0, -1, -1, -1]
_DX = [1, 1, 0, -1, -1, -1, 0, 1]

K0SQ = float(np.cos(np.pi / 8) ** 2)      # 0.85355339059
K1SQ = float(np.cos(3 * np.pi / 8) ** 2)  # 0.14644660941

WALL_BYTE = 56.0   # fp8 e4m3 encoding of 1.0, read as u8

LANES = [(0, i, c) for i, c in enumerate((1, 2, 5, 6))] + \
        [(1, i, c) for i, c in enumerate((0, 7))]   # (word-slot, lane, channel)

USE_BCAST_CVALS = True
REPEAT = 1  # profiling knob: emit the whole pipeline N times
SKIP = set()  # timing-attribution knob: subset of {"A","B","C"}

_cache = {}


def _interior(t):
    return t[:, 1:1 + S, 1:1 + W]


def _view(t, dy, dx):
    # value of neighbor at (y+dy, x+dx) for each interior pixel
    return t[:, 1 + dy:1 + S + dy, 1 + dx:1 + W + dx]


def _interior4(t):
    return t[:, 1:1 + S, :, 1:1 + W]


def _view4(t, dy, dx):
    return t[:, 1 + dy:1 + S + dy, :, 1 + dx:1 + W + dx]


def _fp8_lane(t, k, lane):
    """fp8 strided view [P,6,Wp] of lane `lane` of word-slot `k` of [P,6,2,Wp] i32."""
    b = t[:].bitcast(dt.float8e4)                 # [P, 6, 2, 4*Wp]
    b = b.rearrange("p s k (c l) -> p s k c l", l=4)
    return b[:, :, k, :, lane]


def _u8_lane(t, k, lane):
    b = t[:].bitcast(dt.uint8)
    b = b.rearrange("p s k (c l) -> p s k c l", l=4)
    return b[:, :, k, :, lane]


class _Emit:
    def __init__(self, nk):
        self.nk = nk  # 3x3 conv kernel (already /18)
        nc = self.nc = bacc.Bacc()
        self.win = nc.declare_dram_parameter("w", [NB, 8, 512, 512], dt.float32, isOutput=False)
        self.wout = nc.declare_dram_parameter("o", [NB, 8, 512, 512], dt.float32, isOutput=True)

    def build(self):
        nc = self.nc
        with TileContext(nc) as tc:
            self.tc = tc
            with (
                tc.tile_pool(name="pconst", bufs=1) as pconst,
                tc.tile_pool(name="pvel", bufs=2) as pvel,      # f32 padded [P,6,2,Wp]: vy,vx
                tc.tile_pool(name="pw32", bufs=3) as pw32,      # f32 padded [P,6,Wp]
                tc.tile_pool(name="pwAB", bufs=2) as pwAB,      # int32 padded [P,6,2,Wp]: payload fp8 lanes
                tc.tile_pool(name="pmask", bufs=8) as pmask,    # u8 tight [P,S,W]: equ8
                tc.tile_pool(name="pf32t", bufs=3) as pf32t,    # f32 tight [P,S,W]
                tc.tile_pool(name="pu8t", bufs=12) as pu8t,     # u8 tight: mask algebra + Wm2
                tc.tile_pool(name="pE1", bufs=1) as pE1,        # u8 padded: eqm1
                tc.tile_pool(name="pM8", bufs=1) as pM8,        # u8 padded: match mask
                tc.tile_pool(name="pEE", bufs=1) as pEE,        # u8 padded: emptyE
                tc.tile_pool(name="psw", bufs=1) as psw,        # u8 tight: swaps
            ):
                self.pconst, self.pw32, self.pvel = pconst, pw32, pvel
                self.pwAB = pwAB
                self.pmask, self.pf32t, self.pu8t = pmask, pf32t, pu8t
                self.pE1, self.pM8, self.pEE, self.psw = pE1, pM8, pEE, psw
                if USE_BCAST_CVALS:
                    self.cvals = pconst.tile([P, 9, 4], dt.uint8, tag="cvals", name="cvals")
                    for v in range(9):
                        nc.vector.memset(self.cvals[:, v:v + 1, :], v)
                else:
                    self.cvals = pconst.tile([P, 9 * S, W], dt.uint8, tag="cvals", name="cvals")
                    for v in range(9):
                        nc.vector.memset(self.cvals[:, v * S:(v + 1) * S, :], v)
                for _r in range(REPEAT):
                    for b in range(NB):
                        st = self.image_load(b)
                        for n in range(2):
                            self.image_iter(st, n)
                        self.image_final(b, st)
        nc.compile()
        return nc

    def cval(self, v):
        if USE_BCAST_CVALS:
            return self.cvals[:, v:v + 1, 0:1].to_broadcast([P, S, W])
        return self.cvals[:, v * S:(v + 1) * S, :]

    def cval_like(self, v, sub):
        return self.cvals[:, v:v + 1, 0:1].to_broadcast(list(sub.shape))

    def u8(self):
        return self.pu8t.tile([P, S, W], dt.uint8, tag="bft", name="bft")

    # ---------- halo helpers ----------

    @staticmethod
    def _cslice(t, slots, cs):
        # column slice helper, rank-agnostic ([P,6,Wp] or [P,6,2,Wp])
        if len(t.shape) == 4:
            return t[:, slots, :, cs]
        return t[:, slots, cs]

    def fill_xcols(self, t, slots=slice(1, 5), engine=None):
        nc = self.nc
        e = engine or nc.vector
        lo_o, lo_i = self._cslice(t, slots, slice(0, 1)), self._cslice(t, slots, slice(W, W + 1))
        hi_o, hi_i = self._cslice(t, slots, slice(Wp - 1, Wp)), self._cslice(t, slots, slice(1, 2))
        if e is nc.scalar:
            e.copy(out=lo_o, in_=lo_i)
            e.copy(out=hi_o, in_=hi_i)
        else:
            e.tensor_copy(out=lo_o, in_=lo_i)
            e.tensor_copy(out=hi_o, in_=hi_i)

    def fill_xcol_side(self, t, dx, slots=slice(1, 5)):
        # tiny column copies ride the (idle) scalar queue, not DVE
        nc = self.nc
        if dx > 0:
            nc.scalar.copy(out=self._cslice(t, slots, slice(Wp - 1, Wp)),
                           in_=self._cslice(t, slots, slice(1, 2)))
        elif dx < 0:
            nc.scalar.copy(out=self._cslice(t, slots, slice(0, 1)),
                           in_=self._cslice(t, slots, slice(W, W + 1)))

    def fill_yhalo(self, t, hi, zero_edge=False, dma=None):
        # compute ops need 32-aligned partition bases: zero the whole halo slot
        # first, then let the partition-shift DMA overwrite all but the edge row
        # (issued from the SP queue to keep ACT free for compute)
        nc = self.nc
        dma = dma or nc.sync
        r4 = len(t.shape) == 4

        def sl(ps, s):
            return t[ps, s, :, :] if r4 else t[ps, s, :]

        if hi:
            if zero_edge:
                nc.vector.memset(sl(slice(0, P), 5), 0)
            dma.dma_start(out=sl(slice(0, P - 1), 5), in_=sl(slice(1, P), 1))
            if not zero_edge:
                dma.dma_start(out=sl(slice(P - 1, P), 5), in_=sl(slice(0, 1), 1))
        else:
            if zero_edge:
                nc.vector.memset(sl(slice(0, P), 0), 0)
            dma.dma_start(out=sl(slice(1, P), 0), in_=sl(slice(0, P - 1), 4))
            if not zero_edge:
                dma.dma_start(out=sl(slice(0, 1), 0), in_=sl(slice(P - 1, P), 4))

    def fill_halos(self, t):
        self.fill_xcols(t)
        self.fill_yhalo(t, hi=True)
        self.fill_yhalo(t, hi=False)

    # ---------- DRAM loads (iter 0) ----------

    def _load_padded_f32(self, b, c, t):
        nc = self.nc
        d = self.win[b, c].rearrange("(p k) x -> p k x", k=S)  # [128, 4, 512]
        nc.sync.dma_start(out=t[:, 1:1 + S, 1:1 + W], in_=d)
        nc.sync.dma_start(out=t[1:P, 0, 1:1 + W], in_=d[0:P - 1, S - 1, :])
        nc.sync.dma_start(out=t[0:1, 0, 1:1 + W], in_=d[P - 1:P, S - 1, :])
        nc.sync.dma_start(out=t[0:P - 1, 5, 1:1 + W], in_=d[1:P, 0, :])
        nc.sync.dma_start(out=t[P - 1:P, 5, 1:1 + W], in_=d[0:1, 0, :])

    def load_vel(self, b):
        t = self.pvel.tile([P, 6, 2, Wp], dt.float32, tag="vel", name="vel")
        for i, c in enumerate((3, 4)):
            d = self.win[b, c].rearrange("(p k) x -> p k x", k=S)  # [128, 4, 512]
            nc = self.nc
            nc.sync.dma_start(out=t[:, 1:1 + S, i, 1:1 + W], in_=d)
            nc.sync.dma_start(out=t[1:P, 0, i, 1:1 + W], in_=d[0:P - 1, S - 1, :])
            nc.sync.dma_start(out=t[0:1, 0, i, 1:1 + W], in_=d[P - 1:P, S - 1, :])
            nc.sync.dma_start(out=t[0:P - 1, 5, i, 1:1 + W], in_=d[1:P, 0, :])
            nc.sync.dma_start(out=t[P - 1:P, 5, i, 1:1 + W], in_=d[0:1, 0, :])
        self.fill_xcols(t, slots=slice(0, 6))
        return t

    def _load_word(self, b):
        nc = self.nc
        t = self.pwAB.tile([P, 6, 2, Wp], dt.int32, tag="wd", name="wd")
        nc.vector.memset(t[:], 0)
        for k, lane, ch in LANES:
            stg = self.pw32.tile([P, 6, Wp], dt.float32, tag="w32", name="stg")
            self._load_padded_f32(b, ch, stg)
            self.fill_xcols(stg, slots=slice(0, 6), engine=nc.scalar)
            nc.scalar.copy(out=_fp8_lane(t, k, lane), in_=stg[:])  # cast f32->fp8
        return t

    # ---------- phase A: direction masks ----------

    def phase_A(self, vel, wAB, thresh_sq):
        """Returns Wm2[a] (u8 tight, = want-move-a & enough & shifted-empty) and emits them."""
        nc = self.nc
        vy = vel[:, 1:1 + S, 0, 1:1 + W]
        vx = vel[:, 1:1 + S, 1, 1:1 + W]

        t1 = self.pf32t.tile([P, S, W], dt.float32, tag="f32t", name="t1")
        nc.scalar.activation(t1[:], vy, Act.Square)
        t2 = self.pf32t.tile([P, S, W], dt.float32, tag="f32t", name="t2")
        nc.scalar.activation(t2[:], vx, Act.Square)
        nc.vector.tensor_tensor(out=t1[:], in0=t1[:], in1=t2[:], op=Alu.add)  # m2
        m2k = t1
        t3 = self.pf32t.tile([P, S, W], dt.float32, tag="f32t", name="t3")
        nc.scalar.activation(t3[:], t1[:], Act.Sqrt)
        nc.vector.scalar_tensor_tensor(out=t3[:], in0=t3[:], scalar=0.002, in1=t1[:],
                                       op0=Alu.mult, op1=Alu.add)             # magp2 - 1e-6
        # enough = (m2>th) & (E != 1); must read m2 before T0/T1 reuse t1
        e_b = _u8_lane(wAB, 1, 0)
        wallok = self.u8()
        nc.vector.tensor_scalar(out=wallok[:], in0=e_b[:, 1:1 + S, 1:1 + W],
                                scalar1=WALL_BYTE, scalar2=None, op0=Alu.not_equal)
        en = self.u8()
        nc.vector.scalar_tensor_tensor(out=en[:], in0=m2k[:], scalar=thresh_sq,
                                       in1=wallok[:], op0=Alu.is_gt, op1=Alu.mult)
        nc.scalar.activation(t1[:], t3[:], Act.Copy, bias=1e-6 * K0SQ, scale=K0SQ)  # T0
        g0 = self.u8()
        nc.vector.tensor_tensor(out=g0[:], in0=t2[:], in1=t1[:], op=Alu.is_le)
        nc.scalar.activation(t1[:], t3[:], Act.Copy, bias=1e-6 * K1SQ, scale=K1SQ)  # T1
        g1 = self.u8()
        nc.vector.tensor_tensor(out=g1[:], in0=t2[:], in1=t1[:], op=Alu.is_le)
        zb = self.u8()
        nc.vector.tensor_scalar(out=zb[:], in0=vx, scalar1=0.0, scalar2=None, op0=Alu.is_le)
        # band masks via the g-delta algebra:
        #   band2 (a=2/6) = (zb|g1) - (zb&~g1) == g1 exactly
        #   gd = g0&~g1 splits into d3 = zb&gd (a=3/5) and d1 = gd-d3 (a=1/7)
        gd = self.u8()
        nc.vector.tensor_tensor(out=gd[:], in0=g0[:], in1=g1[:], op=Alu.subtract)
        d3 = self.u8()
        nc.vector.tensor_tensor(out=d3[:], in0=gd[:], in1=zb[:], op=Alu.mult)
        d1 = self.u8()
        nc.vector.tensor_tensor(out=d1[:], in0=gd[:], in1=d3[:], op=Alu.subtract)
        u0 = self.u8()
        nc.vector.tensor_tensor(out=u0[:], in0=zb[:], in1=g0[:], op=Alu.max)
        nc.scalar.activation(u0[:], u0[:], Act.Copy, bias=1.0, scale=-1.0)   # nu0, band a=0
        nc.scalar.activation(g0[:], g0[:], Act.Copy, bias=1.0, scale=-1.0)   # ng0
        u3 = self.u8()
        nc.vector.tensor_tensor(out=u3[:], in0=zb[:], in1=g0[:], op=Alu.mult)  # zb&~g0, band a=4
        s1 = self.u8()
        nc.vector.tensor_scalar(out=s1[:], in0=vy, scalar1=0.0, scalar2=None, op0=Alu.is_lt)
        nc.vector.tensor_tensor(out=s1[:], in0=s1[:], in1=en[:], op=Alu.mult)
        s0 = self.u8()
        nc.vector.tensor_tensor(out=s0[:], in0=en[:], in1=s1[:], op=Alu.subtract)
        # emptyE u8 padded, from E byte lane of wordB, all slots incl halos
        emptyE = self.pEE.tile([P, 6, Wp], dt.uint8, tag="EE", name="emptyE")
        nc.vector.tensor_scalar(out=emptyE[:], in0=e_b, scalar1=0.0, scalar2=None, op0=Alu.is_equal)

        Wm2 = [None] * 8

        def emit_w(a, f0, f1):
            eng = nc.vector
            tmp = self.pu8t.tile([P, S, W], dt.uint8, tag="bft", name="wtmp")
            eng.tensor_tensor(out=tmp[:], in0=f0[:], in1=f1[:], op=Alu.mult)
            m = self.pu8t.tile([P, S, W], dt.uint8, tag="bft", name="wm")
            eng.tensor_tensor(out=m[:], in0=tmp[:], in1=_view(emptyE, _DY[a], _DX[a]), op=Alu.mult)
            Wm2[a] = m

        emit_w(1, d1, s0)
        emit_w(7, d1, s1)
        emit_w(0, u0, en)
        emit_w(2, g1, s0)
        emit_w(6, g1, s1)
        emit_w(3, d3, s0)
        emit_w(5, d3, s1)
        emit_w(4, u3, en)
        return Wm2

    # ---------- phase B: sequential swap resolution ----------

    # fragment splits over TIGHT [P,S,W] tiles; torus wraps become direct reads
    @staticmethod
    def _ssplit(dy):
        """[(out_slots, in_slots, edge)] for reading src[s+dy]; edge reads hrow."""
        if dy == 0:
            return [(slice(0, S), slice(0, S), False)]
        if dy > 0:
            return [(slice(0, S - 1), slice(1, S), False), (slice(S - 1, S), slice(0, 1), True)]
        return [(slice(1, S), slice(0, S - 1), False), (slice(0, 1), slice(S - 1, S), True)]

    @staticmethod
    def _xsplit(dx):
        """[(out_cols, in_cols)] for reading src[x+dx] with mod-512 wrap."""
        if dx == 0:
            return [(slice(0, W), slice(0, W))]
        if dx > 0:
            return [(slice(0, W - 1), slice(1, W)), (slice(W - 1, W), slice(0, 1))]
        return [(slice(1, W), slice(0, W - 1)), (slice(0, 1), slice(W - 1, W))]

    def _hrow_dma(self, dst, src, dy):
        """dst[p,0,:] = src[p+dy, slot (0 if dy>0 else S-1), :] with torus wrap."""
        nc = self.nc
        if dy > 0:
            nc.sync.dma_start(out=dst[0:P - 1, 0, :], in_=src[1:P, 0, :])
            nc.sync.dma_start(out=dst[P - 1:P, 0, :], in_=src[0:1, 0, :])
        else:
            nc.sync.dma_start(out=dst[1:P, 0, :], in_=src[0:P - 1, S - 1, :])
            nc.sync.dma_start(out=dst[0:1, 0, :], in_=src[P - 1:P, S - 1, :])

    def phase_B(self, Wm2):
        nc = self.nc
        swaps = self.psw.tile([P, S, W], dt.uint8, tag="swaps", name="swaps")
        nc.vector.memset(swaps[:], 8)
        for a in range(8):
            dy, dx = _DY[a], _DX[a]
            a4 = (a + 4) % 8
            dy4, dx4 = -dy, -dx
            if a == 0:
                M8 = Wm2[0]
            else:
                E1 = self.pE1.tile([P, S, W], dt.uint8, tag="E1", name="E1")
                nc.vector.tensor_scalar(out=E1[:], in0=swaps[:], scalar1=8.0,
                                        scalar2=None, op0=Alu.is_equal)
                if dy != 0:
                    ehrow = self.pE1.tile([P, 1, W], dt.uint8, tag="ehrow", name="ehrow")
                    self._hrow_dma(ehrow, E1, dy)
                mtmp = self.pu8t.tile([P, S, W], dt.uint8, tag="bft", name="mtmp")
                nc.vector.tensor_tensor(out=mtmp[:], in0=Wm2[a][:], in1=E1[:], op=Alu.mult)
                M8 = self.pM8.tile([P, S, W], dt.uint8, tag="M8", name="M8")
                for so, si, edge in self._ssplit(dy):
                    srce = ehrow if edge else E1
                    sie = slice(0, 1) if edge else si
                    for xo, xi in self._xsplit(dx):
                        nc.vector.tensor_tensor(out=M8[:, so, xo], in0=mtmp[:, so, xo],
                                                in1=srce[:, sie, xi], op=Alu.mult)
            if dy4 != 0:
                mhrow = self.pM8.tile([P, 1, W], dt.uint8, tag="mhrow", name="mhrow")
                self._hrow_dma(mhrow, M8, dy4)
            nc.vector.copy_predicated(out=swaps[:], mask=M8[:], data=self.cval(a))
            for so, si, edge in self._ssplit(dy4):
                srce = mhrow if edge else M8
                sie = slice(0, 1) if edge else si
                for xo, xi in self._xsplit(dx4):
                    sub = swaps[:, so, xo]
                    nc.vector.copy_predicated(out=sub, mask=srce[:, sie, xi],
                                              data=self.cval_like(a4, sub))
        return swaps

    # ---------- phase C: gather ----------

    def phase_C(self, swaps, streams):
        nc = self.nc
        equ8 = []
        for a in range(8):
            m = self.pmask.tile([P, S, 1, W], dt.uint8, tag="m8", name="equ")
            nc.vector.tensor_scalar(out=m[:, :, 0, :], in0=swaps[:], scalar1=float(a),
                                    scalar2=None, op0=Alu.is_equal)
            equ8.append(m)
        news = []
        for t, kind in streams:
            if kind == "vel":
                nt = self.pvel.tile([P, 6, 2, Wp], dt.float32, tag="vel", name="nvel")
                nc.scalar.copy(out=_interior4(nt), in_=_interior4(t))
            else:
                nt = self.pwAB.tile([P, 6, 2, Wp], dt.int32, tag="wd", name="nwAB")
                nc.sync.dma_start(out=_interior4(nt), in_=_interior4(t))
            for a in range(8):
                nc.vector.copy_predicated(
                    out=_interior4(nt),
                    mask=equ8[a][:].to_broadcast([P, S, 2, W]),
                    data=_view4(t, _DY[a], _DX[a]))
            news.append(nt)
        return news

    # ---------- final conv ----------

    def conv_channel(self, vf, out_tight):
        """out = conv3x3(vf, nk) + 0.5*vf (zero padding); vf padded with zeroed edges."""
        nc = self.nc
        nk = self.nk
        uniform = bool(np.allclose(nk, nk[0, 0]))
        kys = [0] if uniform else [0, 1, 2]
        tmps = []
        for ky in kys:
            tp = self.pw32.tile([P, 6, Wp], dt.float32, tag="w32", name="convtp")
            if uniform:
                nc.vector.tensor_tensor(out=_interior(tp), in0=_view(vf, 0, -1),
                                        in1=_view(vf, 0, 0), op=Alu.add)
                nc.vector.tensor_tensor(out=_interior(tp), in0=_interior(tp),
                                        in1=_view(vf, 0, 1), op=Alu.add)
            else:
                nc.scalar.mul(_interior(tp), _view(vf, 0, 0), float(nk[ky, 1]))
                nc.vector.scalar_tensor_tensor(out=_interior(tp), in0=_view(vf, 0, -1),
                                               scalar=float(nk[ky, 0]), in1=_interior(tp),
                                               op0=Alu.mult, op1=Alu.add)
                nc.vector.scalar_tensor_tensor(out=_interior(tp), in0=_view(vf, 0, 1),
                                               scalar=float(nk[ky, 2]), in1=_interior(tp),
                                               op0=Alu.mult, op1=Alu.add)
            self.fill_yhalo(tp, hi=True, zero_edge=True)
            self.fill_yhalo(tp, hi=False, zero_edge=True)
            tmps.append(tp)
        if uniform:
            tmps = [tmps[0]] * 3
        acc = self.pf32t.tile([P, S, W], dt.float32, tag="f32t", name="acc")
        nc.vector.tensor_tensor(out=acc[:], in0=_view(tmps[0], -1, 0),
                                in1=_view(tmps[1], 0, 0), op=Alu.add)
        nc.vector.tensor_tensor(out=acc[:], in0=acc[:], in1=_view(tmps[2], 1, 0), op=Alu.add)
        vfh = self.pf32t.tile([P, S, W], dt.float32, tag="f32t", name="vfh")
        nc.scalar.mul(vfh[:], _interior(vf), 0.5)
        scale = float(nk[0, 0]) if uniform else 1.0
        nc.vector.scalar_tensor_tensor(out=out_tight[:], in0=acc[:], scalar=scale,
                                       in1=vfh[:], op0=Alu.mult, op1=Alu.add)

    # ---------- per-image program ----------

    def image_load(self, b):
        st = {}
        st["vel"] = self.load_vel(b)
        st["wAB"] = self._load_word(b)
        return st

    def image_iter(self, st, n):
        nc = self.nc
        vel, wAB = st["vel"], st["wAB"]
        thresh_sq = 1.0 if n == 0 else 4.0
        if "A" in SKIP:
            Wm2 = []
            for _ in range(8):
                m = self.pu8t.tile([P, S, W], dt.uint8, tag="bft", name="wm")
                nc.vector.memset(m[:], 0)
                Wm2.append(m)
        else:
            Wm2 = self.phase_A(vel, wAB, thresh_sq)
        if "B" in SKIP:
            swaps = self.psw.tile([P, S, W], dt.uint8, tag="swaps", name="swaps")
            nc.vector.memset(swaps[:], 8)
        else:
            swaps = self.phase_B(Wm2)
        streams = [(wAB, "wAB"), (vel, "vel")]
        if "C" in SKIP:
            nAB, nv = wAB, vel
        else:
            nAB, nv = self.phase_C(swaps, streams)
        for i in range(2):
            old_i = vel[:, 1:1 + S, i, 1:1 + W]
            new_i = nv[:, 1:1 + S, i, 1:1 + W]
            vh = self.pf32t.tile([P, S, W], dt.float32, tag="f32t", name="vh")
            nc.scalar.mul(vh[:], old_i, 0.5)
            nc.vector.scalar_tensor_tensor(out=new_i, in0=new_i,
                                           scalar=0.5, in1=vh[:], op0=Alu.mult, op1=Alu.add)
        st["wAB"], st["vel"] = nAB, nv
        if n == 0:
            for t in (nAB, nv):
                self.fill_halos(t)

    def image_final(self, b, st):
        nc = self.nc
        vel, wAB = st["vel"], st["wAB"]

        # final: vel *= 0.95, zero-padded halos, 3x3 smoothing conv
        for c, i in ((3, 0), (4, 1)):
            vf = self.pw32.tile([P, 6, Wp], dt.float32, tag="w32", name="convstg")
            nc.scalar.mul(_interior(vf), vel[:, 1:1 + S, i, 1:1 + W], 0.95)
            nc.vector.memset(vf[:, 1:5, 0:1], 0)
            nc.vector.memset(vf[:, 1:5, Wp - 1:Wp], 0)
            self.fill_yhalo(vf, hi=True, zero_edge=True)
            self.fill_yhalo(vf, hi=False, zero_edge=True)
            ot = self.pf32t.tile([P, S, W], dt.float32, tag="f32t", name="convout")
            self.conv_channel(vf, ot)
            nc.sync.dma_start(out=self.wout[b, c].rearrange("(p k) x -> p k x", k=S), in_=ot[:])

        for k, lane, ch in LANES:
            view = _fp8_lane(wAB, k, lane)
            stg = self.pf32t.tile([P, S, W], dt.float32, tag="f32t", name="ostg")
            nc.scalar.copy(out=stg[:], in_=view[:, 1:1 + S, 1:1 + W])
            nc.sync.dma_start(out=self.wout[b, ch].rearrange("(p k) x -> p k x", k=S),
                              in_=stg[:])


def _build(nk):
    return _Emit(nk).build()


def kernel(world, rand_movement=None, rand_interact=None, rand_element=None,
           neighbor_kernel=None, **_kw):
    world = np.ascontiguousarray(np.asarray(world, dtype=np.float32))
    nk = np.asarray(neighbor_kernel, dtype=np.float32).reshape(3, 3) / 18.0
    key = nk.tobytes()
    nc = _cache.get(key)
    if nc is None:
        nc = _cache[key] = _build(nk)
    in_maps = [{"w": world[NB * i:NB * (i + 1)]} for i in range(NCORES)]
    res = run_bass_kernel_spmd(nc, in_maps, list(range(NCORES))).results
    return np.concatenate([r["o"] for r in res], axis=0)


# revision 26
# speedup vs baseline: 1.1074x; 1.1074x over previous
"""Trainium2 Bass kernel for nn_BehaviorVelocity (velocity-driven swap sim + smoothing).

Sharding: data-parallel over batch B=16 across 8 cores (2 images/core, no collectives).

Layout per 512x512 image: partition p holds rows 4p..4p+3 as free-dim "slots".
Padded field = [128, 6 slots, 514 cols]:
  slot 0 = row 4p-1 (y-halo lo), slots 1..4 = rows 4p..4p+3, slot 5 = row 4p+4 (y-halo hi)
  col 0 = x=511 (wrap), cols 1..512 = x=0..511, col 513 = x=0 (wrap)
y-halos: partition-shift SBUF DMAs (+1-row torus wrap DMA). x-halos: tiny strided copies.
All spatial shifts then become free-dim AP offsets (compute ops must start at partition 0).

Channels 3,4 (vy,vx) stay f32 (decision precision). Payload channels are fp8(e4m3)
packed into containers so each copy_predicated moves more channels per cycle:
  wordA int32 = (ch1, ch2, ch5, ch6) fp8 bytes
  wordB uint16 = (ch0, ch7) fp8 bytes; ch0 (element id) is byte 0, compared as u8
  (0.0 -> 0x00, 1.0 -> 0x38).

Sector selection replicates floor(8*arccos-angle+0.5) via threshold compares in the
squared domain:  vx <= K*(mag+0.001)  <=>  (vx<=0) or (vx^2 <= K^2*magp2)  with
magp2 = m2 + 0.002*mag + 1e-6, so the ACT-sqrt LUT error only enters the tiny
0.002*mag term (~5e-8 boundary shift instead of ~3e-5).
"""

import sys

sys.path.insert(0, "/opt/trn_rl_repo")

import numpy as np

import concourse.bacc as bacc
import concourse.mybir as mybir
from concourse.tile import TileContext
from concourse.bass_utils import run_bass_kernel_spmd

dt = mybir.dt
Alu = mybir.AluOpType
Act = mybir.ActivationFunctionType

P = 128          # partitions
S = 4            # row-slots per partition (512 rows / 128)
W = 512
Wp = W + 2       # 514 with x-halo cols
NB = 2           # batch images per core
NCORES = 8

_DY = [0, 1, 1, 1, 0, -1, -1, -1]
_DX = [1, 1, 0, -1, -1, -1, 0, 1]

K0SQ = float(np.cos(np.pi / 8) ** 2)      # 0.85355339059
K1SQ = float(np.cos(3 * np.pi / 8) ** 2)  # 0.14644660941

WALL_BYTE = 56.0   # fp8 e4m3 encoding of 1.0, read as u8

LANES = [(0, i, c) for i, c in enumerate((1, 2, 5, 6))] + \
        [(1, i, c) for i, c in enumerate((0, 7))]   # (word-slot, lane, channel)

USE_BCAST_CVALS = True
REPEAT = 1  # profiling knob: emit the whole pipeline N times
SKIP = set()  # timing-attribution knob: subset of {"A","B","C"}

_cache = {}


def _interior(t):
    return t[:, 1:1 + S, 1:1 + W]


def _view(t, dy, dx):
    # value of neighbor at (y+dy, x+dx) for each interior pixel
    return t[:, 1 + dy:1 + S + dy, 1 + dx:1 + W + dx]


def _interior4(t):
    return t[:, 1:1 + S, :, 1:1 + W]


def _view4(t, dy, dx):
    return t[:, 1 + dy:1 + S + dy, :, 1 + dx:1 + W + dx]


def _fp8_lane(t, k, lane):
    """fp8 strided view [P,6,Wp] of lane `lane` of word-slot `k` of [P,6,2,Wp] i32."""
    b = t[:].bitcast(dt.float8e4)                 # [P, 6, 2, 4*Wp]
    b = b.rearrange("p s k (c l) -> p s k c l", l=4)
    return b[:, :, k, :, lane]


def _u8_lane(t, k, lane):
    b = t[:].bitcast(dt.uint8)
    b = b.rearrange("p s k (c l) -> p s k c l", l=4)
    return b[:, :, k, :, lane]


class _Emit:
    def __init__(self, nk):
        self.nk = nk  # 3x3 conv kernel (already /18)
        nc = self.nc = bacc.Bacc()
        self.win = nc.declare_dram_parameter("w", [NB, 8, 512, 512], dt.float32, isOutput=False)
        self.wout = nc.declare_dram_parameter("o", [NB, 8, 512, 512], dt.float32, isOutput=True)

    def build(self):
        nc = self.nc
        with TileContext(nc) as tc:
            self.tc = tc
            with (
                tc.tile_pool(name="pconst", bufs=1) as pconst,
                tc.tile_pool(name="pvel", bufs=2) as pvel,      # f32 padded [P,6,2,Wp]: vy,vx
                tc.tile_pool(name="pw32", bufs=2) as pw32,      # f32 padded [P,6,Wp]
                tc.tile_pool(name="pwAB", bufs=2) as pwAB,      # int32 padded [P,6,2,Wp]: payload fp8 lanes
                tc.tile_pool(name="pmask", bufs=9) as pmask,    # u8 tight [P,S,W]: equ8
                tc.tile_pool(name="pf32t", bufs=4) as pf32t,    # f32 tight [P,S,W]
                tc.tile_pool(name="pu8t", bufs=12) as pu8t,     # u8 tight: mask algebra + Wm2
                tc.tile_pool(name="pE1", bufs=1) as pE1,        # u8 padded: eqm1
                tc.tile_pool(name="pM8", bufs=1) as pM8,        # u8 padded: match mask
                tc.tile_pool(name="pEE", bufs=1) as pEE,        # u8 padded: emptyE
                tc.tile_pool(name="psw", bufs=2) as psw,        # u8 tight: swaps
            ):
                self.pconst, self.pw32, self.pvel = pconst, pw32, pvel
                self.pwAB = pwAB
                self.pmask, self.pf32t, self.pu8t = pmask, pf32t, pu8t
                self.pE1, self.pM8, self.pEE, self.psw = pE1, pM8, pEE, psw
                if USE_BCAST_CVALS:
                    self.cvals = pconst.tile([P, 9, 4], dt.uint8, tag="cvals", name="cvals")
                    for v in range(9):
                        nc.vector.memset(self.cvals[:, v:v + 1, :], v)
                else:
                    self.cvals = pconst.tile([P, 9 * S, W], dt.uint8, tag="cvals", name="cvals")
                    for v in range(9):
                        nc.vector.memset(self.cvals[:, v * S:(v + 1) * S, :], v)
                for _r in range(REPEAT):
                    for b in range(NB):
                        st = self.image_load(b)
                        for n in range(2):
                            self.image_iter(st, n)
                        self.image_final(b, st)
        nc.compile()
        return nc

    def cval(self, v):
        if USE_BCAST_CVALS:
            return self.cvals[:, v:v + 1, 0:1].to_broadcast([P, S, W])
        return self.cvals[:, v * S:(v + 1) * S, :]

    def cval_like(self, v, sub):
        return self.cvals[:, v:v + 1, 0:1].to_broadcast(list(sub.shape))

    def u8(self):
        return self.pu8t.tile([P, S, W], dt.uint8, tag="bft", name="bft")

    # ---------- halo helpers ----------

    @staticmethod
    def _cslice(t, slots, cs):
        # column slice helper, rank-agnostic ([P,6,Wp] or [P,6,2,Wp])
        if len(t.shape) == 4:
            return t[:, slots, :, cs]
        return t[:, slots, cs]

    def fill_xcols(self, t, slots=slice(1, 5), engine=None):
        nc = self.nc
        e = engine or nc.vector
        lo_o, lo_i = self._cslice(t, slots, slice(0, 1)), self._cslice(t, slots, slice(W, W + 1))
        hi_o, hi_i = self._cslice(t, slots, slice(Wp - 1, Wp)), self._cslice(t, slots, slice(1, 2))
        if e is nc.scalar:
            e.copy(out=lo_o, in_=lo_i)
            e.copy(out=hi_o, in_=hi_i)
        else:
            e.tensor_copy(out=lo_o, in_=lo_i)
            e.tensor_copy(out=hi_o, in_=hi_i)

    def fill_xcol_side(self, t, dx, slots=slice(1, 5)):
        # tiny column copies ride the (idle) scalar queue, not DVE
        nc = self.nc
        if dx > 0:
            nc.scalar.copy(out=self._cslice(t, slots, slice(Wp - 1, Wp)),
                           in_=self._cslice(t, slots, slice(1, 2)))
        elif dx < 0:
            nc.scalar.copy(out=self._cslice(t, slots, slice(0, 1)),
                           in_=self._cslice(t, slots, slice(W, W + 1)))

    def fill_yhalo(self, t, hi, zero_edge=False, dma=None):
        # compute ops need 32-aligned partition bases: zero the whole halo slot
        # first, then let the partition-shift DMA overwrite all but the edge row
        # (issued from the SP queue to keep ACT free for compute)
        nc = self.nc
        dma = dma or nc.sync
        r4 = len(t.shape) == 4

        def sl(ps, s):
            return t[ps, s, :, :] if r4 else t[ps, s, :]

        if hi:
            if zero_edge:
                nc.vector.memset(sl(slice(0, P), 5), 0)
            dma.dma_start(out=sl(slice(0, P - 1), 5), in_=sl(slice(1, P), 1))
            if not zero_edge:
                dma.dma_start(out=sl(slice(P - 1, P), 5), in_=sl(slice(0, 1), 1))
        else:
            if zero_edge:
                nc.vector.memset(sl(slice(0, P), 0), 0)
            dma.dma_start(out=sl(slice(1, P), 0), in_=sl(slice(0, P - 1), 4))
            if not zero_edge:
                dma.dma_start(out=sl(slice(0, 1), 0), in_=sl(slice(P - 1, P), 4))

    def fill_halos(self, t):
        self.fill_xcols(t)
        self.fill_yhalo(t, hi=True)
        self.fill_yhalo(t, hi=False)

    # ---------- DRAM loads (iter 0) ----------

    def _load_padded_f32(self, b, c, t):
        nc = self.nc
        d = self.win[b, c].rearrange("(p k) x -> p k x", k=S)  # [128, 4, 512]
        nc.sync.dma_start(out=t[:, 1:1 + S, 1:1 + W], in_=d)
        nc.sync.dma_start(out=t[1:P, 0, 1:1 + W], in_=d[0:P - 1, S - 1, :])
        nc.sync.dma_start(out=t[0:1, 0, 1:1 + W], in_=d[P - 1:P, S - 1, :])
        nc.sync.dma_start(out=t[0:P - 1, 5, 1:1 + W], in_=d[1:P, 0, :])
        nc.sync.dma_start(out=t[P - 1:P, 5, 1:1 + W], in_=d[0:1, 0, :])

    def load_vel(self, b):
        t = self.pvel.tile([P, 6, 2, Wp], dt.float32, tag="vel", name="vel")
        for i, c in enumerate((3, 4)):
            d = self.win[b, c].rearrange("(p k) x -> p k x", k=S)  # [128, 4, 512]
            nc = self.nc
            nc.sync.dma_start(out=t[:, 1:1 + S, i, 1:1 + W], in_=d)
            nc.sync.dma_start(out=t[1:P, 0, i, 1:1 + W], in_=d[0:P - 1, S - 1, :])
            nc.sync.dma_start(out=t[0:1, 0, i, 1:1 + W], in_=d[P - 1:P, S - 1, :])
            nc.sync.dma_start(out=t[0:P - 1, 5, i, 1:1 + W], in_=d[1:P, 0, :])
            nc.sync.dma_start(out=t[P - 1:P, 5, i, 1:1 + W], in_=d[0:1, 0, :])
        self.fill_xcols(t, slots=slice(0, 6))
        return t

    def _load_word(self, b):
        nc = self.nc
        t = self.pwAB.tile([P, 6, 2, Wp], dt.int32, tag="wd", name="wd")
        nc.vector.memset(t[:], 0)
        for k, lane, ch in LANES:
            stg = self.pw32.tile([P, 6, Wp], dt.float32, tag="w32", name="stg")
            self._load_padded_f32(b, ch, stg)
            self.fill_xcols(stg, slots=slice(0, 6), engine=nc.scalar)
            nc.scalar.copy(out=_fp8_lane(t, k, lane), in_=stg[:])  # cast f32->fp8
        return t

    # ---------- phase A: direction masks ----------

    def phase_A(self, vel, wAB, thresh_sq):
        """Returns Wm2[a] (u8 tight, = want-move-a & enough & shifted-empty) and emits them."""
        nc = self.nc
        vy = vel[:, 1:1 + S, 0, 1:1 + W]
        vx = vel[:, 1:1 + S, 1, 1:1 + W]

        t1 = self.pf32t.tile([P, S, W], dt.float32, tag="f32t", name="t1")
        nc.scalar.activation(t1[:], vy, Act.Square)
        t2 = self.pf32t.tile([P, S, W], dt.float32, tag="f32t", name="t2")
        nc.scalar.activation(t2[:], vx, Act.Square)
        nc.vector.tensor_tensor(out=t1[:], in0=t1[:], in1=t2[:], op=Alu.add)  # m2
        m2k = t1
        t3 = self.pf32t.tile([P, S, W], dt.float32, tag="f32t", name="t3")
        nc.scalar.activation(t3[:], t1[:], Act.Sqrt)
        nc.vector.scalar_tensor_tensor(out=t3[:], in0=t3[:], scalar=0.002, in1=t1[:],
                                       op0=Alu.mult, op1=Alu.add)             # magp2 - 1e-6
        # enough = (m2>th) & (E != 1); must read m2 before T0/T1 reuse t1
        e_b = _u8_lane(wAB, 1, 0)
        wallok = self.u8()
        nc.vector.tensor_scalar(out=wallok[:], in0=e_b[:, 1:1 + S, 1:1 + W],
                                scalar1=WALL_BYTE, scalar2=None, op0=Alu.not_equal)
        en = self.u8()
        nc.vector.scalar_tensor_tensor(out=en[:], in0=m2k[:], scalar=thresh_sq,
                                       in1=wallok[:], op0=Alu.is_gt, op1=Alu.mult)
        nc.scalar.activation(t1[:], t3[:], Act.Copy, bias=1e-6 * K0SQ, scale=K0SQ)  # T0
        g0 = self.u8()
        nc.vector.tensor_tensor(out=g0[:], in0=t2[:], in1=t1[:], op=Alu.is_le)
        nc.scalar.activation(t1[:], t3[:], Act.Copy, bias=1e-6 * K1SQ, scale=K1SQ)  # T1
        g1 = self.u8()
        nc.vector.tensor_tensor(out=g1[:], in0=t2[:], in1=t1[:], op=Alu.is_le)
        zb = self.u8()
        nc.vector.tensor_scalar(out=zb[:], in0=vx, scalar1=0.0, scalar2=None, op0=Alu.is_le)
        # band masks via the g-delta algebra:
        #   band2 (a=2/6) = (zb|g1) - (zb&~g1) == g1 exactly
        #   gd = g0&~g1 splits into d3 = zb&gd (a=3/5) and d1 = gd-d3 (a=1/7)
        gd = self.u8()
        nc.vector.tensor_tensor(out=gd[:], in0=g0[:], in1=g1[:], op=Alu.subtract)
        d3 = self.u8()
        nc.vector.tensor_tensor(out=d3[:], in0=gd[:], in1=zb[:], op=Alu.mult)
        d1 = self.u8()
        nc.vector.tensor_tensor(out=d1[:], in0=gd[:], in1=d3[:], op=Alu.subtract)
        u0 = self.u8()
        nc.vector.tensor_tensor(out=u0[:], in0=zb[:], in1=g0[:], op=Alu.max)
        nc.scalar.activation(u0[:], u0[:], Act.Copy, bias=1.0, scale=-1.0)   # nu0, band a=0
        nc.scalar.activation(g0[:], g0[:], Act.Copy, bias=1.0, scale=-1.0)   # ng0
        u3 = self.u8()
        nc.vector.tensor_tensor(out=u3[:], in0=zb[:], in1=g0[:], op=Alu.mult)  # zb&~g0, band a=4
        s1 = self.u8()
        nc.vector.tensor_scalar(out=s1[:], in0=vy, scalar1=0.0, scalar2=None, op0=Alu.is_lt)
        nc.vector.tensor_tensor(out=s1[:], in0=s1[:], in1=en[:], op=Alu.mult)
        s0 = self.u8()
        nc.vector.tensor_tensor(out=s0[:], in0=en[:], in1=s1[:], op=Alu.subtract)
        # emptyE u8 padded, from E byte lane of wordB, all slots incl halos
        emptyE = self.pEE.tile([P, 6, Wp], dt.uint8, tag="EE", name="emptyE")
        nc.vector.tensor_scalar(out=emptyE[:], in0=e_b, scalar1=0.0, scalar2=None, op0=Alu.is_equal)

        Wm2 = [None] * 8

        def emit_w(a, f0, f1):
            eng = nc.vector
            tmp = self.pu8t.tile([P, S, W], dt.uint8, tag="bft", name="wtmp")
            eng.tensor_tensor(out=tmp[:], in0=f0[:], in1=f1[:], op=Alu.mult)
            m = self.pu8t.tile([P, S, W], dt.uint8, tag="bft", name="wm")
            eng.tensor_tensor(out=m[:], in0=tmp[:], in1=_view(emptyE, _DY[a], _DX[a]), op=Alu.mult)
            Wm2[a] = m

        emit_w(1, d1, s0)
        emit_w(7, d1, s1)
        emit_w(0, u0, en)
        emit_w(2, g1, s0)
        emit_w(6, g1, s1)
        emit_w(3, d3, s0)
        emit_w(5, d3, s1)
        emit_w(4, u3, en)
        return Wm2

    # ---------- phase B: sequential swap resolution ----------

    # fragment splits over TIGHT [P,S,W] tiles; torus wraps become direct reads
    @staticmethod
    def _ssplit(dy):
        """[(out_slots, in_slots, edge)] for reading src[s+dy]; edge reads hrow."""
        if dy == 0:
            return [(slice(0, S), slice(0, S), False)]
        if dy > 0:
            return [(slice(0, S - 1), slice(1, S), False), (slice(S - 1, S), slice(0, 1), True)]
        return [(slice(1, S), slice(0, S - 1), False), (slice(0, 1), slice(S - 1, S), True)]

    @staticmethod
    def _xsplit(dx):
        """[(out_cols, in_cols)] for reading src[x+dx] with mod-512 wrap."""
        if dx == 0:
            return [(slice(0, W), slice(0, W))]
        if dx > 0:
            return [(slice(0, W - 1), slice(1, W)), (slice(W - 1, W), slice(0, 1))]
        return [(slice(1, W), slice(0, W - 1)), (slice(0, 1), slice(W - 1, W))]

    def _hrow_dma(self, dst, src, dy):
        """dst[p,0,:] = src[p+dy, slot (0 if dy>0 else S-1), :] with torus wrap."""
        nc = self.nc
        if dy > 0:
            nc.sync.dma_start(out=dst[0:P - 1, 0, :], in_=src[1:P, 0, :])
            nc.sync.dma_start(out=dst[P - 1:P, 0, :], in_=src[0:1, 0, :])
        else:
            nc.sync.dma_start(out=dst[1:P, 0, :], in_=src[0:P - 1, S - 1, :])
            nc.sync.dma_start(out=dst[0:1, 0, :], in_=src[P - 1:P, S - 1, :])

    def phase_B(self, Wm2):
        nc = self.nc
        swaps = self.psw.tile([P, S, W], dt.uint8, tag="swaps", name="swaps")
        nc.vector.memset(swaps[:], 8)
        for a in range(8):
            dy, dx = _DY[a], _DX[a]
            a4 = (a + 4) % 8
            dy4, dx4 = -dy, -dx
            if a == 0:
                M8 = Wm2[0]
            else:
                E1 = self.pE1.tile([P, S, W], dt.uint8, tag="E1", name="E1")
                nc.vector.tensor_scalar(out=E1[:], in0=swaps[:], scalar1=8.0,
                                        scalar2=None, op0=Alu.is_equal)
                if dy != 0:
                    ehrow = self.pE1.tile([P, 1, W], dt.uint8, tag="ehrow", name="ehrow")
                    self._hrow_dma(ehrow, E1, dy)
                mtmp = self.pu8t.tile([P, S, W], dt.uint8, tag="bft", name="mtmp")
                nc.vector.tensor_tensor(out=mtmp[:], in0=Wm2[a][:], in1=E1[:], op=Alu.mult)
                M8 = self.pM8.tile([P, S, W], dt.uint8, tag="M8", name="M8")
                for so, si, edge in self._ssplit(dy):
                    srce = ehrow if edge else E1
                    sie = slice(0, 1) if edge else si
                    for xo, xi in self._xsplit(dx):
                        nc.vector.tensor_tensor(out=M8[:, so, xo], in0=mtmp[:, so, xo],
                                                in1=srce[:, sie, xi], op=Alu.mult)
            if dy4 != 0:
                mhrow = self.pM8.tile([P, 1, W], dt.uint8, tag="mhrow", name="mhrow")
                self._hrow_dma(mhrow, M8, dy4)
            nc.vector.copy_predicated(out=swaps[:], mask=M8[:], data=self.cval(a))
            for so, si, edge in self._ssplit(dy4):
                srce = mhrow if edge else M8
                sie = slice(0, 1) if edge else si
                for xo, xi in self._xsplit(dx4):
                    sub = swaps[:, so, xo]
                    nc.vector.copy_predicated(out=sub, mask=srce[:, sie, xi],
                                              data=self.cval_like(a4, sub))
        return swaps

    # ---------- phase C: gather ----------

    def phase_C(self, swaps, streams):
        nc = self.nc
        equ8 = []
        for a in range(8):
            m = self.pmask.tile([P, S, 1, W], dt.uint8, tag="m8", name="equ")
            nc.vector.tensor_scalar(out=m[:, :, 0, :], in0=swaps[:], scalar1=float(a),
                                    scalar2=None, op0=Alu.is_equal)
            equ8.append(m)
        news = []
        for t, kind in streams:
            if kind == "vel":
                nt = self.pvel.tile([P, 6, 2, Wp], dt.float32, tag="vel", name="nvel")
                nc.scalar.copy(out=_interior4(nt), in_=_interior4(t))
            else:
                nt = self.pwAB.tile([P, 6, 2, Wp], dt.int32, tag="wd", name="nwAB")
                nc.sync.dma_start(out=_interior4(nt), in_=_interior4(t))
            for a in range(8):
                nc.vector.copy_predicated(
                    out=_interior4(nt),
                    mask=equ8[a][:].to_broadcast([P, S, 2, W]),
                    data=_view4(t, _DY[a], _DX[a]))
            news.append(nt)
        return news

    # ---------- final conv ----------

    def conv_channel(self, vf, out_tight):
        """out = conv3x3(vf, nk) + 0.5*vf (zero padding); vf padded with zeroed edges."""
        nc = self.nc
        nk = self.nk
        uniform = bool(np.allclose(nk, nk[0, 0]))
        kys = [0] if uniform else [0, 1, 2]
        tmps = []
        for ky in kys:
            tp = self.pw32.tile([P, 6, Wp], dt.float32, tag="w32", name="convtp")
            if uniform:
                nc.vector.tensor_tensor(out=_interior(tp), in0=_view(vf, 0, -1),
                                        in1=_view(vf, 0, 0), op=Alu.add)
                nc.vector.tensor_tensor(out=_interior(tp), in0=_interior(tp),
                                        in1=_view(vf, 0, 1), op=Alu.add)
            else:
                nc.scalar.mul(_interior(tp), _view(vf, 0, 0), float(nk[ky, 1]))
                nc.vector.scalar_tensor_tensor(out=_interior(tp), in0=_view(vf, 0, -1),
                                               scalar=float(nk[ky, 0]), in1=_interior(tp),
                                               op0=Alu.mult, op1=Alu.add)
                nc.vector.scalar_tensor_tensor(out=_interior(tp), in0=_view(vf, 0, 1),
                                               scalar=float(nk[ky, 2]), in1=_interior(tp),
                                               op0=Alu.mult, op1=Alu.add)
            self.fill_yhalo(tp, hi=True, zero_edge=True)
            self.fill_yhalo(tp, hi=False, zero_edge=True)
            tmps.append(tp)
        if uniform:
            tmps = [tmps[0]] * 3
        acc = self.pf32t.tile([P, S, W], dt.float32, tag="f32t", name="acc")
        nc.vector.tensor_tensor(out=acc[:], in0=_view(tmps[0], -1, 0),
                                in1=_view(tmps[1], 0, 0), op=Alu.add)
        nc.vector.tensor_tensor(out=acc[:], in0=acc[:], in1=_view(tmps[2], 1, 0), op=Alu.add)
        vfh = self.pf32t.tile([P, S, W], dt.float32, tag="f32t", name="vfh")
        nc.scalar.mul(vfh[:], _interior(vf), 0.5)
        scale = float(nk[0, 0]) if uniform else 1.0
        nc.vector.scalar_tensor_tensor(out=out_tight[:], in0=acc[:], scalar=scale,
                                       in1=vfh[:], op0=Alu.mult, op1=Alu.add)

    # ---------- per-image program ----------

    def image_load(self, b):
        st = {}
        st["vel"] = self.load_vel(b)
        st["wAB"] = self._load_word(b)
        return st

    def image_iter(self, st, n):
        nc = self.nc
        vel, wAB = st["vel"], st["wAB"]
        thresh_sq = 1.0 if n == 0 else 4.0
        if "A" in SKIP:
            Wm2 = []
            for _ in range(8):
                m = self.pu8t.tile([P, S, W], dt.uint8, tag="bft", name="wm")
                nc.vector.memset(m[:], 0)
                Wm2.append(m)
        else:
            Wm2 = self.phase_A(vel, wAB, thresh_sq)
        if "B" in SKIP:
            swaps = self.psw.tile([P, S, W], dt.uint8, tag="swaps", name="swaps")
            nc.vector.memset(swaps[:], 8)
        else:
            swaps = self.phase_B(Wm2)
        streams = [(wAB, "wAB"), (vel, "vel")]
        if "C" in SKIP:
            nAB, nv = wAB, vel
        else:
            nAB, nv = self.phase_C(swaps, streams)
        for i in range(2):
            old_i = vel[:, 1:1 + S, i, 1:1 + W]
            new_i = nv[:, 1:1 + S, i, 1:1 + W]
            vh = self.pf32t.tile([P, S, W], dt.float32, tag="f32t", name="vh")
            nc.scalar.mul(vh[:], old_i, 0.5)
            nc.vector.scalar_tensor_tensor(out=new_i, in0=new_i,
                                           scalar=0.5, in1=vh[:], op0=Alu.mult, op1=Alu.add)
        st["wAB"], st["vel"] = nAB, nv
        if n == 0:
            for t in (nAB, nv):
                self.fill_halos(t)

    def image_final(self, b, st):
        nc = self.nc
        vel, wAB = st["vel"], st["wAB"]

        # final: vel *= 0.95, zero-padded halos, 3x3 smoothing conv
        for c, i in ((3, 0), (4, 1)):
            vf = self.pw32.tile([P, 6, Wp], dt.float32, tag="w32", name="convstg")
            nc.scalar.mul(_interior(vf), vel[:, 1:1 + S, i, 1:1 + W], 0.95)
            nc.vector.memset(vf[:, 1:5, 0:1], 0)
            nc.vector.memset(vf[:, 1:5, Wp - 1:Wp], 0)
            self.fill_yhalo(vf, hi=True, zero_edge=True)
            self.fill_yhalo(vf, hi=False, zero_edge=True)
            ot = self.pf32t.tile([P, S, W], dt.float32, tag="f32t", name="convout")
            self.conv_channel(vf, ot)
            nc.sync.dma_start(out=self.wout[b, c].rearrange("(p k) x -> p k x", k=S), in_=ot[:])

        for k, lane, ch in LANES:
            view = _fp8_lane(wAB, k, lane)
            stg = self.pf32t.tile([P, S, W], dt.float32, tag="f32t", name="ostg")
            nc.scalar.copy(out=stg[:], in_=view[:, 1:1 + S, 1:1 + W])
            nc.sync.dma_start(out=self.wout[b, ch].rearrange("(p k) x -> p k x", k=S),
                              in_=stg[:])


def _build(nk):
    return _Emit(nk).build()


def kernel(world, rand_movement=None, rand_interact=None, rand_element=None,
           neighbor_kernel=None, **_kw):
    world = np.ascontiguousarray(np.asarray(world, dtype=np.float32))
    nk = np.asarray(neighbor_kernel, dtype=np.float32).reshape(3, 3) / 18.0
    key = nk.tobytes()
    nc = _cache.get(key)
    if nc is None:
        nc = _cache[key] = _build(nk)
    in_maps = [{"w": world[NB * i:NB * (i + 1)]} for i in range(NCORES)]
    res = run_bass_kernel_spmd(nc, in_maps, list(range(NCORES))).results
    return np.concatenate([r["o"] for r in res], axis=0)


# revision 27
# speedup vs baseline: 1.4728x; 1.3299x over previous
"""Trainium2 Bass kernel for nn_BehaviorVelocity (velocity-driven swap sim + smoothing).

Sharding: data-parallel over batch B=16 across 8 cores (2 images/core, no collectives).

Layout per 512x512 image: partition p holds rows 4p..4p+3 as free-dim "slots".
Padded field = [128, 6 slots, 514 cols]:
  slot 0 = row 4p-1 (y-halo lo), slots 1..4 = rows 4p..4p+3, slot 5 = row 4p+4 (y-halo hi)
  col 0 = x=511 (wrap), cols 1..512 = x=0..511, col 513 = x=0 (wrap)
y-halos: partition-shift SBUF DMAs (+1-row torus wrap DMA). x-halos: tiny strided copies.
All spatial shifts then become free-dim AP offsets (compute ops must start at partition 0).

Channels 3,4 (vy,vx) stay f32 (decision precision). Payload channels are fp8(e4m3)
packed into containers so each copy_predicated moves more channels per cycle:
  wordA int32 = (ch1, ch2, ch5, ch6) fp8 bytes
  wordB uint16 = (ch0, ch7) fp8 bytes; ch0 (element id) is byte 0, compared as u8
  (0.0 -> 0x00, 1.0 -> 0x38).

Sector selection replicates floor(8*arccos-angle+0.5) via threshold compares in the
squared domain:  vx <= K*(mag+0.001)  <=>  (vx<=0) or (vx^2 <= K^2*magp2)  with
magp2 = m2 + 0.002*mag + 1e-6, so the ACT-sqrt LUT error only enters the tiny
0.002*mag term (~5e-8 boundary shift instead of ~3e-5).
"""

import sys

sys.path.insert(0, "/opt/trn_rl_repo")

import numpy as np

import concourse.bacc as bacc
import concourse.mybir as mybir
from concourse.tile import TileContext
from concourse.bass_utils import run_bass_kernel_spmd

dt = mybir.dt
Alu = mybir.AluOpType
Act = mybir.ActivationFunctionType

P = 128          # partitions
S = 4            # row-slots per partition (512 rows / 128)
W = 512
Wp = W + 2       # 514 with x-halo cols
NB = 2           # batch images per core
NCORES = 8

_DY = [0, 1, 1, 1, 0, -1, -1, -1]
_DX = [1, 1, 0, -1, -1, -1, 0, 1]

K0SQ = float(np.cos(np.pi / 8) ** 2)      # 0.85355339059
K1SQ = float(np.cos(3 * np.pi / 8) ** 2)  # 0.14644660941

WALL_BYTE = 56.0   # fp8 e4m3 encoding of 1.0, read as u8

LANES = [(0, i, c) for i, c in enumerate((1, 2, 5, 6))] + \
        [(1, i, c) for i, c in enumerate((0, 7))]   # (word-slot, lane, channel)

USE_BCAST_CVALS = True
REPEAT = 1  # profiling knob: emit the whole pipeline N times
SKIP = set()  # timing-attribution knob: subset of {"A","B","C"}

_cache = {}


def _interior(t):
    return t[:, 1:1 + S, 1:1 + W]


def _view(t, dy, dx):
    # value of neighbor at (y+dy, x+dx) for each interior pixel
    return t[:, 1 + dy:1 + S + dy, 1 + dx:1 + W + dx]


def _interior4(t):
    return t[:, 1:1 + S, :, 1:1 + W]


def _view4(t, dy, dx):
    return t[:, 1 + dy:1 + S + dy, :, 1 + dx:1 + W + dx]


def _fp8_lane(t, k, lane):
    """fp8 strided view [P,6,Wp] of lane `lane` of word-slot `k` of [P,6,2,Wp] i32."""
    b = t[:].bitcast(dt.float8e4)                 # [P, 6, 2, 4*Wp]
    b = b.rearrange("p s k (c l) -> p s k c l", l=4)
    return b[:, :, k, :, lane]


def _u8_lane(t, k, lane):
    b = t[:].bitcast(dt.uint8)
    b = b.rearrange("p s k (c l) -> p s k c l", l=4)
    return b[:, :, k, :, lane]


class _Emit:
    def __init__(self, nk):
        self.nk = nk  # 3x3 conv kernel (already /18)
        nc = self.nc = bacc.Bacc()
        self.win = nc.declare_dram_parameter("w", [NB, 8, 512, 512], dt.float32, isOutput=False)
        self.wout = nc.declare_dram_parameter("o", [NB, 8, 512, 512], dt.float32, isOutput=True)

    def build(self):
        nc = self.nc
        with TileContext(nc) as tc:
            self.tc = tc
            with (
                tc.tile_pool(name="pconst", bufs=1) as pconst,
                tc.tile_pool(name="pvel", bufs=2) as pvel,      # f32 padded [P,6,2,Wp]: vy,vx
                tc.tile_pool(name="pw32", bufs=2) as pw32,      # f32 padded [P,6,Wp]
                tc.tile_pool(name="pwAB", bufs=2) as pwAB,      # int32 padded [P,6,2,Wp]: payload fp8 lanes
                tc.tile_pool(name="pmask", bufs=9) as pmask,    # u8 tight [P,S,W]: equ8
                tc.tile_pool(name="pf32t", bufs=4) as pf32t,    # f32 tight [P,S,W]
                tc.tile_pool(name="pu8t", bufs=12) as pu8t,     # u8 tight: mask algebra + Wm2
                tc.tile_pool(name="pE1", bufs=1) as pE1,        # u8 padded: eqm1
                tc.tile_pool(name="pM8", bufs=1) as pM8,        # u8 padded: match mask
                tc.tile_pool(name="pEE", bufs=1) as pEE,        # u8 padded: emptyE
                tc.tile_pool(name="psw", bufs=2) as psw,        # u8 tight: swaps
            ):
                self.pconst, self.pw32, self.pvel = pconst, pw32, pvel
                self.pwAB = pwAB
                self.pmask, self.pf32t, self.pu8t = pmask, pf32t, pu8t
                self.pE1, self.pM8, self.pEE, self.psw = pE1, pM8, pEE, psw
                if USE_BCAST_CVALS:
                    self.cvals = pconst.tile([P, 9, 4], dt.uint8, tag="cvals", name="cvals")
                    for v in range(9):
                        nc.vector.memset(self.cvals[:, v:v + 1, :], v)
                else:
                    self.cvals = pconst.tile([P, 9 * S, W], dt.uint8, tag="cvals", name="cvals")
                    for v in range(9):
                        nc.vector.memset(self.cvals[:, v * S:(v + 1) * S, :], v)
                for _r in range(REPEAT):
                    for b in range(NB):
                        st = self.image_load(b)
                        for n in range(2):
                            self.image_iter(st, n)
                        self.image_final(b, st)
        nc.compile()
        return nc

    def cval(self, v):
        if USE_BCAST_CVALS:
            return self.cvals[:, v:v + 1, 0:1].to_broadcast([P, S, W])
        return self.cvals[:, v * S:(v + 1) * S, :]

    def cval_like(self, v, sub):
        return self.cvals[:, v:v + 1, 0:1].to_broadcast(list(sub.shape))

    def u8(self):
        return self.pu8t.tile([P, S, W], dt.uint8, tag="bft", name="bft")

    # ---------- halo helpers ----------

    @staticmethod
    def _cslice(t, slots, cs):
        # column slice helper, rank-agnostic ([P,6,Wp] or [P,6,2,Wp])
        if len(t.shape) == 4:
            return t[:, slots, :, cs]
        return t[:, slots, cs]

    def fill_xcols(self, t, slots=slice(1, 5), engine=None):
        nc = self.nc
        e = engine or nc.vector
        lo_o, lo_i = self._cslice(t, slots, slice(0, 1)), self._cslice(t, slots, slice(W, W + 1))
        hi_o, hi_i = self._cslice(t, slots, slice(Wp - 1, Wp)), self._cslice(t, slots, slice(1, 2))
        if e is nc.scalar:
            e.copy(out=lo_o, in_=lo_i)
            e.copy(out=hi_o, in_=hi_i)
        else:
            e.tensor_copy(out=lo_o, in_=lo_i)
            e.tensor_copy(out=hi_o, in_=hi_i)

    def fill_xcol_side(self, t, dx, slots=slice(1, 5)):
        # tiny column copies ride the (idle) scalar queue, not DVE
        nc = self.nc
        if dx > 0:
            nc.scalar.copy(out=self._cslice(t, slots, slice(Wp - 1, Wp)),
                           in_=self._cslice(t, slots, slice(1, 2)))
        elif dx < 0:
            nc.scalar.copy(out=self._cslice(t, slots, slice(0, 1)),
                           in_=self._cslice(t, slots, slice(W, W + 1)))

    def fill_yhalo(self, t, hi, zero_edge=False, dma=None):
        # compute ops need 32-aligned partition bases: zero the whole halo slot
        # first, then let the partition-shift DMA overwrite all but the edge row
        # (issued from the SP queue to keep ACT free for compute)
        nc = self.nc
        dma = dma or nc.sync
        r4 = len(t.shape) == 4

        def sl(ps, s):
            return t[ps, s, :, :] if r4 else t[ps, s, :]

        if hi:
            if zero_edge:
                nc.vector.memset(sl(slice(0, P), 5), 0)
            dma.dma_start(out=sl(slice(0, P - 1), 5), in_=sl(slice(1, P), 1))
            if not zero_edge:
                dma.dma_start(out=sl(slice(P - 1, P), 5), in_=sl(slice(0, 1), 1))
        else:
            if zero_edge:
                nc.vector.memset(sl(slice(0, P), 0), 0)
            dma.dma_start(out=sl(slice(1, P), 0), in_=sl(slice(0, P - 1), 4))
            if not zero_edge:
                dma.dma_start(out=sl(slice(0, 1), 0), in_=sl(slice(P - 1, P), 4))

    def fill_halos(self, t):
        self.fill_xcols(t)
        self.fill_yhalo(t, hi=True)
        self.fill_yhalo(t, hi=False)

    # ---------- DRAM loads (iter 0) ----------

    def _load_padded_f32(self, b, c, t):
        # interior only; tile halos are refilled from SBUF afterwards
        nc = self.nc
        d = self.win[b, c].rearrange("(p k) x -> p k x", k=S)  # [128, 4, 512]
        nc.sync.dma_start(out=t[:, 1:1 + S, 1:1 + W], in_=d)

    def load_vel(self, b):
        t = self.pvel.tile([P, 6, 2, Wp], dt.float32, tag="vel", name="vel")
        for i, c in enumerate((3, 4)):
            d = self.win[b, c].rearrange("(p k) x -> p k x", k=S)  # [128, 4, 512]
            self.nc.sync.dma_start(out=t[:, 1:1 + S, i, 1:1 + W], in_=d)
        self.fill_halos(t)
        return t

    def _load_word(self, b):
        nc = self.nc
        t = self.pwAB.tile([P, 6, 2, Wp], dt.int32, tag="wd", name="wd")
        nc.vector.memset(t[:], 0)
        for k, lane, ch in LANES:
            stg = self.pw32.tile([P, 6, Wp], dt.float32, tag="w32", name="stg")
            self._load_padded_f32(b, ch, stg)
            lane_v = _fp8_lane(t, k, lane)
            nc.scalar.copy(out=lane_v[:, 1:1 + S, 1:1 + W],
                           in_=stg[:, 1:1 + S, 1:1 + W])  # cast f32->fp8
        self.fill_halos(t)
        return t

    # ---------- phase A: direction masks ----------

    def phase_A(self, vel, wAB, thresh_sq):
        """Returns Wm2[a] (u8 tight, = want-move-a & enough & shifted-empty) and emits them."""
        nc = self.nc
        vy = vel[:, 1:1 + S, 0, 1:1 + W]
        vx = vel[:, 1:1 + S, 1, 1:1 + W]

        t1 = self.pf32t.tile([P, S, W], dt.float32, tag="f32t", name="t1")
        nc.scalar.activation(t1[:], vy, Act.Square)
        t2 = self.pf32t.tile([P, S, W], dt.float32, tag="f32t", name="t2")
        nc.scalar.activation(t2[:], vx, Act.Square)
        nc.vector.tensor_tensor(out=t1[:], in0=t1[:], in1=t2[:], op=Alu.add)  # m2
        m2k = t1
        t3 = self.pf32t.tile([P, S, W], dt.float32, tag="f32t", name="t3")
        nc.scalar.activation(t3[:], t1[:], Act.Sqrt)
        nc.vector.scalar_tensor_tensor(out=t3[:], in0=t3[:], scalar=0.002, in1=t1[:],
                                       op0=Alu.mult, op1=Alu.add)             # magp2 - 1e-6
        # enough = (m2>th) & (E != 1); must read m2 before T0/T1 reuse t1
        e_b = _u8_lane(wAB, 1, 0)
        wallok = self.u8()
        nc.vector.tensor_scalar(out=wallok[:], in0=e_b[:, 1:1 + S, 1:1 + W],
                                scalar1=WALL_BYTE, scalar2=None, op0=Alu.not_equal)
        en = self.u8()
        nc.vector.scalar_tensor_tensor(out=en[:], in0=m2k[:], scalar=thresh_sq,
                                       in1=wallok[:], op0=Alu.is_gt, op1=Alu.mult)
        nc.scalar.activation(t1[:], t3[:], Act.Copy, bias=1e-6 * K0SQ, scale=K0SQ)  # T0
        g0 = self.u8()
        nc.vector.tensor_tensor(out=g0[:], in0=t2[:], in1=t1[:], op=Alu.is_le)
        nc.scalar.activation(t1[:], t3[:], Act.Copy, bias=1e-6 * K1SQ, scale=K1SQ)  # T1
        g1 = self.u8()
        nc.vector.tensor_tensor(out=g1[:], in0=t2[:], in1=t1[:], op=Alu.is_le)
        zb = self.u8()
        nc.vector.tensor_scalar(out=zb[:], in0=vx, scalar1=0.0, scalar2=None, op0=Alu.is_le)
        # band masks via the g-delta algebra:
        #   band2 (a=2/6) = (zb|g1) - (zb&~g1) == g1 exactly
        #   gd = g0&~g1 splits into d3 = zb&gd (a=3/5) and d1 = gd-d3 (a=1/7)
        gd = self.u8()
        nc.vector.tensor_tensor(out=gd[:], in0=g0[:], in1=g1[:], op=Alu.subtract)
        d3 = self.u8()
        nc.vector.tensor_tensor(out=d3[:], in0=gd[:], in1=zb[:], op=Alu.mult)
        d1 = self.u8()
        nc.vector.tensor_tensor(out=d1[:], in0=gd[:], in1=d3[:], op=Alu.subtract)
        u0 = self.u8()
        nc.vector.tensor_tensor(out=u0[:], in0=zb[:], in1=g0[:], op=Alu.max)
        nc.scalar.activation(u0[:], u0[:], Act.Copy, bias=1.0, scale=-1.0)   # nu0, band a=0
        nc.scalar.activation(g0[:], g0[:], Act.Copy, bias=1.0, scale=-1.0)   # ng0
        u3 = self.u8()
        nc.vector.tensor_tensor(out=u3[:], in0=zb[:], in1=g0[:], op=Alu.mult)  # zb&~g0, band a=4
        s1 = self.u8()
        nc.vector.tensor_scalar(out=s1[:], in0=vy, scalar1=0.0, scalar2=None, op0=Alu.is_lt)
        nc.vector.tensor_tensor(out=s1[:], in0=s1[:], in1=en[:], op=Alu.mult)
        s0 = self.u8()
        nc.vector.tensor_tensor(out=s0[:], in0=en[:], in1=s1[:], op=Alu.subtract)
        # emptyE u8 padded, from E byte lane of wordB, all slots incl halos
        emptyE = self.pEE.tile([P, 6, Wp], dt.uint8, tag="EE", name="emptyE")
        nc.vector.tensor_scalar(out=emptyE[:], in0=e_b, scalar1=0.0, scalar2=None, op0=Alu.is_equal)

        Wm2 = [None] * 8

        def emit_w(a, f0, f1):
            eng = nc.vector
            tmp = self.pu8t.tile([P, S, W], dt.uint8, tag="bft", name="wtmp")
            eng.tensor_tensor(out=tmp[:], in0=f0[:], in1=f1[:], op=Alu.mult)
            m = self.pu8t.tile([P, S, W], dt.uint8, tag="bft", name="wm")
            eng.tensor_tensor(out=m[:], in0=tmp[:], in1=_view(emptyE, _DY[a], _DX[a]), op=Alu.mult)
            Wm2[a] = m

        emit_w(1, d1, s0)
        emit_w(7, d1, s1)
        emit_w(0, u0, en)
        emit_w(2, g1, s0)
        emit_w(6, g1, s1)
        emit_w(3, d3, s0)
        emit_w(5, d3, s1)
        emit_w(4, u3, en)
        return Wm2

    # ---------- phase B: sequential swap resolution ----------

    # fragment splits over TIGHT [P,S,W] tiles; torus wraps become direct reads
    @staticmethod
    def _ssplit(dy):
        """[(out_slots, in_slots, edge)] for reading src[s+dy]; edge reads hrow."""
        if dy == 0:
            return [(slice(0, S), slice(0, S), False)]
        if dy > 0:
            return [(slice(0, S - 1), slice(1, S), False), (slice(S - 1, S), slice(0, 1), True)]
        return [(slice(1, S), slice(0, S - 1), False), (slice(0, 1), slice(S - 1, S), True)]

    @staticmethod
    def _xsplit(dx):
        """[(out_cols, in_cols)] for reading src[x+dx] with mod-512 wrap."""
        if dx == 0:
            return [(slice(0, W), slice(0, W))]
        if dx > 0:
            return [(slice(0, W - 1), slice(1, W)), (slice(W - 1, W), slice(0, 1))]
        return [(slice(1, W), slice(0, W - 1)), (slice(0, 1), slice(W - 1, W))]

    def _hrow_dma(self, dst, src, dy):
        """dst[p,0,:] = src[p+dy, slot (0 if dy>0 else S-1), :] with torus wrap."""
        nc = self.nc
        if dy > 0:
            nc.sync.dma_start(out=dst[0:P - 1, 0, :], in_=src[1:P, 0, :])
            nc.sync.dma_start(out=dst[P - 1:P, 0, :], in_=src[0:1, 0, :])
        else:
            nc.sync.dma_start(out=dst[1:P, 0, :], in_=src[0:P - 1, S - 1, :])
            nc.sync.dma_start(out=dst[0:1, 0, :], in_=src[P - 1:P, S - 1, :])

    def phase_B(self, Wm2):
        nc = self.nc
        swaps = self.psw.tile([P, S, W], dt.uint8, tag="swaps", name="swaps")
        nc.vector.memset(swaps[:], 8)
        for a in range(8):
            dy, dx = _DY[a], _DX[a]
            a4 = (a + 4) % 8
            dy4, dx4 = -dy, -dx
            if a == 0:
                M8 = Wm2[0]
            else:
                E1 = self.pE1.tile([P, S, W], dt.uint8, tag="E1", name="E1")
                nc.vector.tensor_scalar(out=E1[:], in0=swaps[:], scalar1=8.0,
                                        scalar2=None, op0=Alu.is_equal)
                if dy != 0:
                    ehrow = self.pE1.tile([P, 1, W], dt.uint8, tag="ehrow", name="ehrow")
                    self._hrow_dma(ehrow, E1, dy)
                mtmp = self.pu8t.tile([P, S, W], dt.uint8, tag="bft", name="mtmp")
                nc.vector.tensor_tensor(out=mtmp[:], in0=Wm2[a][:], in1=E1[:], op=Alu.mult)
                M8 = self.pM8.tile([P, S, W], dt.uint8, tag="M8", name="M8")
                for so, si, edge in self._ssplit(dy):
                    srce = ehrow if edge else E1
                    sie = slice(0, 1) if edge else si
                    for xo, xi in self._xsplit(dx):
                        nc.vector.tensor_tensor(out=M8[:, so, xo], in0=mtmp[:, so, xo],
                                                in1=srce[:, sie, xi], op=Alu.mult)
            if dy4 != 0:
                mhrow = self.pM8.tile([P, 1, W], dt.uint8, tag="mhrow", name="mhrow")
                self._hrow_dma(mhrow, M8, dy4)
            nc.vector.copy_predicated(out=swaps[:], mask=M8[:], data=self.cval(a))
            for so, si, edge in self._ssplit(dy4):
                srce = mhrow if edge else M8
                sie = slice(0, 1) if edge else si
                for xo, xi in self._xsplit(dx4):
                    sub = swaps[:, so, xo]
                    nc.vector.copy_predicated(out=sub, mask=srce[:, sie, xi],
                                              data=self.cval_like(a4, sub))
        return swaps

    # ---------- phase C: gather ----------

    def phase_C(self, swaps, streams):
        nc = self.nc
        equ8 = []
        for a in range(8):
            m = self.pmask.tile([P, S, 1, W], dt.uint8, tag="m8", name="equ")
            nc.vector.tensor_scalar(out=m[:, :, 0, :], in0=swaps[:], scalar1=float(a),
                                    scalar2=None, op0=Alu.is_equal)
            equ8.append(m)
        news = []
        for t, kind in streams:
            if kind == "vel":
                nt = self.pvel.tile([P, 6, 2, Wp], dt.float32, tag="vel", name="nvel")
                nc.scalar.copy(out=_interior4(nt), in_=_interior4(t))
            else:
                nt = self.pwAB.tile([P, 6, 2, Wp], dt.int32, tag="wd", name="nwAB")
                nc.sync.dma_start(out=_interior4(nt), in_=_interior4(t))
            for a in range(8):
                nc.vector.copy_predicated(
                    out=_interior4(nt),
                    mask=equ8[a][:].to_broadcast([P, S, 2, W]),
                    data=_view4(t, _DY[a], _DX[a]))
            news.append(nt)
        return news

    # ---------- final conv ----------

    def conv_channel(self, vf, out_tight):
        """out = conv3x3(vf, nk) + 0.5*vf (zero padding); vf padded with zeroed edges."""
        nc = self.nc
        nk = self.nk
        uniform = bool(np.allclose(nk, nk[0, 0]))
        kys = [0] if uniform else [0, 1, 2]
        tmps = []
        for ky in kys:
            tp = self.pw32.tile([P, 6, Wp], dt.float32, tag="w32", name="convtp")
            if uniform:
                nc.vector.tensor_tensor(out=_interior(tp), in0=_view(vf, 0, -1),
                                        in1=_view(vf, 0, 0), op=Alu.add)
                nc.vector.tensor_tensor(out=_interior(tp), in0=_interior(tp),
                                        in1=_view(vf, 0, 1), op=Alu.add)
            else:
                nc.scalar.mul(_interior(tp), _view(vf, 0, 0), float(nk[ky, 1]))
                nc.vector.scalar_tensor_tensor(out=_interior(tp), in0=_view(vf, 0, -1),
                                               scalar=float(nk[ky, 0]), in1=_interior(tp),
                                               op0=Alu.mult, op1=Alu.add)
                nc.vector.scalar_tensor_tensor(out=_interior(tp), in0=_view(vf, 0, 1),
                                               scalar=float(nk[ky, 2]), in1=_interior(tp),
                                               op0=Alu.mult, op1=Alu.add)
            self.fill_yhalo(tp, hi=True, zero_edge=True)
            self.fill_yhalo(tp, hi=False, zero_edge=True)
            tmps.append(tp)
        if uniform:
            tmps = [tmps[0]] * 3
        acc = self.pf32t.tile([P, S, W], dt.float32, tag="f32t", name="acc")
        nc.vector.tensor_tensor(out=acc[:], in0=_view(tmps[0], -1, 0),
                                in1=_view(tmps[1], 0, 0), op=Alu.add)
        nc.vector.tensor_tensor(out=acc[:], in0=acc[:], in1=_view(tmps[2], 1, 0), op=Alu.add)
        vfh = self.pf32t.tile([P, S, W], dt.float32, tag="f32t", name="vfh")
        nc.scalar.mul(vfh[:], _interior(vf), 0.5)
        scale = float(nk[0, 0]) if uniform else 1.0
        nc.vector.scalar_tensor_tensor(out=out_tight[:], in0=acc[:], scalar=scale,
                                       in1=vfh[:], op0=Alu.mult, op1=Alu.add)

    # ---------- per-image program ----------

    def image_load(self, b):
        st = {}
        st["vel"] = self.load_vel(b)
        st["wAB"] = self._load_word(b)
        return st

    def image_iter(self, st, n):
        nc = self.nc
        vel, wAB = st["vel"], st["wAB"]
        thresh_sq = 1.0 if n == 0 else 4.0
        if "A" in SKIP:
            Wm2 = []
            for _ in range(8):
                m = self.pu8t.tile([P, S, W], dt.uint8, tag="bft", name="wm")
                nc.vector.memset(m[:], 0)
                Wm2.append(m)
        else:
            Wm2 = self.phase_A(vel, wAB, thresh_sq)
        if "B" in SKIP:
            swaps = self.psw.tile([P, S, W], dt.uint8, tag="swaps", name="swaps")
            nc.vector.memset(swaps[:], 8)
        else:
            swaps = self.phase_B(Wm2)
        streams = [(wAB, "wAB"), (vel, "vel")]
        if "C" in SKIP:
            nAB, nv = wAB, vel
        else:
            nAB, nv = self.phase_C(swaps, streams)
        for i in range(2):
            old_i = vel[:, 1:1 + S, i, 1:1 + W]
            new_i = nv[:, 1:1 + S, i, 1:1 + W]
            vh = self.pf32t.tile([P, S, W], dt.float32, tag="f32t", name="vh")
            nc.scalar.mul(vh[:], old_i, 0.5)
            nc.vector.scalar_tensor_tensor(out=new_i, in0=new_i,
                                           scalar=0.5, in1=vh[:], op0=Alu.mult, op1=Alu.add)
        st["wAB"], st["vel"] = nAB, nv
        if n == 0:
            for t in (nAB, nv):
                self.fill_halos(t)

    def image_final(self, b, st):
        nc = self.nc
        vel, wAB = st["vel"], st["wAB"]

        # final: vel *= 0.95, zero-padded halos, 3x3 smoothing conv
        for c, i in ((3, 0), (4, 1)):
            vf = self.pw32.tile([P, 6, Wp], dt.float32, tag="w32", name="convstg")
            nc.scalar.mul(_interior(vf), vel[:, 1:1 + S, i, 1:1 + W], 0.95)
            nc.vector.memset(vf[:, 1:5, 0:1], 0)
            nc.vector.memset(vf[:, 1:5, Wp - 1:Wp], 0)
            self.fill_yhalo(vf, hi=True, zero_edge=True)
            self.fill_yhalo(vf, hi=False, zero_edge=True)
            ot = self.pf32t.tile([P, S, W], dt.float32, tag="f32t", name="convout")
            self.conv_channel(vf, ot)
            nc.sync.dma_start(out=self.wout[b, c].rearrange("(p k) x -> p k x", k=S), in_=ot[:])

        for k, lane, ch in LANES:
            view = _fp8_lane(wAB, k, lane)
            stg = self.pf32t.tile([P, S, W], dt.float32, tag="f32t", name="ostg")
            nc.scalar.copy(out=stg[:], in_=view[:, 1:1 + S, 1:1 + W])
            nc.sync.dma_start(out=self.wout[b, ch].rearrange("(p k) x -> p k x", k=S),
                              in_=stg[:])


def _build(nk):
    return _Emit(nk).build()


def kernel(world, rand_movement=None, rand_interact=None, rand_element=None,
           neighbor_kernel=None, **_kw):
    world = np.ascontiguousarray(np.asarray(world, dtype=np.float32))
    nk = np.asarray(neighbor_kernel, dtype=np.float32).reshape(3, 3) / 18.0
    key = nk.tobytes()
    nc = _cache.get(key)
    if nc is None:
        nc = _cache[key] = _build(nk)
    in_maps = [{"w": world[NB * i:NB * (i + 1)]} for i in range(NCORES)]
    res = run_bass_kernel_spmd(nc, in_maps, list(range(NCORES))).results
    return np.concatenate([r["o"] for r in res], axis=0)
